# revision 1
# baseline (speedup 1.0000x reference)
"""Trainium2 Bass kernel for nn_DiseaseModel_mlp (GNN message passing + MLP decoder).

Data parallel over the batch dim: 64 graphs -> 8 NeuronCores x 8 graphs.
All weights replicated. Host does layout staging only (transposes/chunking);
all math from xs/A/cell_emb to score runs on device.

Note: every matmul operand is kept at SBUF base partition 0 — operands at
base 64 hard-crash the device when bases alternate across matmuls.
"""

import numpy as np

ATOM = 34
HID = 256
LATENT = 128
CELLS = 512
B, N = 64, 128
NCORES = 8
G = B // NCORES          # graphs per core = 8

_CACHE = {}


def _build_bass(dbg=0):
    """Build the Bass program (one NeuronCore, SPMD across 8).
    dbg>0: also dump the x state after round dbg (1..3) to out "xdbg"."""
    import concourse.bass as bass
    import concourse.bacc as bacc
    import concourse.mybir as mybir
    import concourse.tile as tile

    f32 = mybir.dt.float32
    AF = mybir.ActivationFunctionType
    OP = mybir.AluOpType
    AX = mybir.AxisListType

    nc = bacc.Bacc(None)

    # ---- DRAM parameters (per-core views; names match in_map keys) ----
    d_x0t = nc.declare_dram_parameter("x0t", [G, ATOM + 1, N], f32, isOutput=False)
    d_m01t = nc.declare_dram_parameter("m01t", [G, N, N], f32, isOutput=False)
    d_cell = nc.declare_dram_parameter("cell", [G, CELLS], f32, isOutput=False)
    d_wg = nc.declare_dram_parameter("wg", [3, ATOM + 1, ATOM], f32, isOutput=False)
    d_a12 = nc.declare_dram_parameter("a12", [3, 2, ATOM], f32, isOutput=False)
    d_wt = nc.declare_dram_parameter("wt", [ATOM + 1, HID], f32, isOutput=False)
    d_wf = nc.declare_dram_parameter("wf", [2, 128, ATOM], f32, isOutput=False)
    d_wf2 = nc.declare_dram_parameter("wf2", [ATOM, LATENT], f32, isOutput=False)
    d_b2n = nc.declare_dram_parameter("b2n", [LATENT, 1], f32, isOutput=False)
    d_w1 = nc.declare_dram_parameter("w1", [5, 128, 128], f32, isOutput=False)
    d_b1 = nc.declare_dram_parameter("b1", [128, 1], f32, isOutput=False)
    d_w2 = nc.declare_dram_parameter("w2", [2, 128, 128], f32, isOutput=False)
    d_b2d = nc.declare_dram_parameter("b2d", [2, 128, 1], f32, isOutput=False)
    d_w3 = nc.declare_dram_parameter("w3", [2, 4, 128, 128], f32, isOutput=False)
    d_b3 = nc.declare_dram_parameter("b3", [4, 128, 1], f32, isOutput=False)
    d_w4 = nc.declare_dram_parameter("w4", [4, 128, 1], f32, isOutput=False)
    d_b4 = nc.declare_dram_parameter("b4", [1, 1], f32, isOutput=False)
    d_i128 = nc.declare_dram_parameter("i128", [128, 128], f32, isOutput=False)
    d_score = nc.declare_dram_parameter("score", [G, 1], f32, isOutput=True)
    _dbgshape = {4: [128, G, N], 5: [ATOM, G, N], 6: [LATENT, G],
                 7: [LATENT, G], 8: [128, 4, G], 9: [128, G],
                 10: [128, 2, G], 11: [128, 4, G], 12: [1, G],
                 13: [1, G]}.get(dbg, [ATOM, G, N])
    d_xdbg = (nc.declare_dram_parameter("xdbg", _dbgshape, f32, isOutput=True)
              if dbg else None)

    with tile.TileContext(nc) as tc:
        with (
            tc.tile_pool(name="singles", bufs=1) as singles,
            tc.tile_pool(name="work", bufs=2) as work,
            tc.tile_pool(name="ps1", bufs=1, space="PSUM") as ps1,
            tc.tile_pool(name="ps2", bufs=1, space="PSUM") as ps2,
        ):
            # ACT table warm-up: first ACT instruction is an Exp so the
            # exp_and_others table loads while input DMAs are in flight.
            warm = singles.tile([1, 1], f32, tag="warm")
            nc.vector.memset(warm, 0.0)
            nc.scalar.activation(out=warm, in_=warm, func=AF.Exp)

            # x state, feature-major: rows 0:34 features, row 34 = ones
            state = singles.tile([ATOM + 1, G, N], f32, tag="state")
            for g in range(G):
                nc.sync.dma_start(out=state[:, g, :], in_=d_x0t[g])

            # initial x kept separately for the d1 residual
            x0td = singles.tile([ATOM, G, N], f32, tag="x0td")
            for g in range(G):
                nc.sync.dma_start(out=x0td[:, g, :], in_=d_x0t[g, 0:ATOM, :])

            m01t_sb = singles.tile([N, G, N], f32, tag="m01t")
            for g in range(G):
                eng = nc.sync if g % 2 == 0 else nc.scalar
                eng.dma_start(out=m01t_sb[:, g, :], in_=d_m01t[g])

            wg_sb = singles.tile([ATOM + 1, 3, ATOM], f32, tag="wg")
            nc.scalar.dma_start(out=wg_sb, in_=d_wg.rearrange("r k d -> k r d"))

            a12b = singles.tile([128, 3, 2, ATOM], f32, tag="a12b")
            a12_bcast = bass.AP(
                tensor=d_a12[:].tensor,
                offset=d_a12[:].offset,
                ap=[[0, 128]] + list(d_a12[:].ap),
            )
            nc.scalar.dma_start(out=a12b, in_=a12_bcast)

            i128_sb = singles.tile([128, 128], f32, tag="i128")
            nc.scalar.dma_start(out=i128_sb, in_=d_i128[:])

            wt_sb = singles.tile([ATOM + 1, 2, 128], f32, tag="wt")
            nc.sync.dma_start(out=wt_sb, in_=d_wt.rearrange("k (h m) -> k h m", h=2))

            wf_sb = singles.tile([128, 2, ATOM], f32, tag="wf")
            nc.sync.dma_start(out=wf_sb, in_=d_wf.rearrange("c k d -> k c d"))

            wf2_sb = singles.tile([ATOM, LATENT], f32, tag="wf2")
            nc.sync.dma_start(out=wf2_sb, in_=d_wf2[:])

            b2n_sb = singles.tile([LATENT, 1], f32, tag="b2n")
            nc.gpsimd.dma_start(out=b2n_sb, in_=d_b2n[:])

            cl_sb = singles.tile([G, CELLS], f32, tag="cell")
            nc.gpsimd.dma_start(out=cl_sb, in_=d_cell[:])

            w1_sb = singles.tile([128, 5, 128], f32, tag="w1")
            nc.gpsimd.dma_start(out=w1_sb, in_=d_w1.rearrange("c k m -> k c m"))
            b1_sb = singles.tile([128, 1], f32, tag="b1")
            nc.gpsimd.dma_start(out=b1_sb, in_=d_b1[:])
            w2_sb = singles.tile([128, 2, 128], f32, tag="w2")
            nc.gpsimd.dma_start(out=w2_sb, in_=d_w2.rearrange("b k m -> k b m"))
            b2d_sb = singles.tile([128, 2], f32, tag="b2d")
            nc.gpsimd.dma_start(out=b2d_sb, in_=d_b2d.rearrange("b k x -> k (b x)"))
            w3_sb = singles.tile([128, 2, 4, 128], f32, tag="w3")
            nc.gpsimd.dma_start(out=w3_sb, in_=d_w3.rearrange("c b k m -> k c b m"))
            b3_sb = singles.tile([128, 4], f32, tag="b3")
            nc.gpsimd.dma_start(out=b3_sb, in_=d_b3.rearrange("b k x -> k (b x)"))
            w4_sb = singles.tile([128, 4], f32, tag="w4")
            nc.gpsimd.dma_start(out=w4_sb, in_=d_w4.rearrange("c k x -> k (c x)"))
            b4_sb = singles.tile([1, 1], f32, tag="b4")
            nc.gpsimd.dma_start(out=b4_sb, in_=d_b4[:])

            ones1 = singles.tile([1, 128], f32, tag="ones1")
            nc.vector.memset(ones1, 1.0)

            # h with a trailing ones column (gives att row-sums for free)
            haug = singles.tile([N, G, ATOM + 1], f32, tag="haug")
            nc.vector.memset(haug[:, :, ATOM], 1.0)

            # ---- GNN rounds ----
            for r in range(3):
                # h = relu(x @ Wg[r] + bg[r]) in node-major layout
                h_ps = ps1.tile([N, G, ATOM], f32, tag="h_ps")
                for g in range(G):
                    nc.tensor.matmul(h_ps[:, g, :], state[:, g, :],
                                     wg_sb[:, r, :], start=True, stop=True)
                nc.vector.tensor_scalar_max(haug[:, :, 0:ATOM], h_ps, 0.0)

                # f_src/f_dst = h @ a1, h @ a2: multiply on GPSIMD, reduce on DVE
                tf = work.tile([N, 2, G, ATOM], f32, tag="tf")
                h_b = haug[:, :, 0:ATOM].unsqueeze(1).to_broadcast([N, 2, G, ATOM])
                a_b = a12b[:, r, :, :].unsqueeze(2).to_broadcast([128, 2, G, ATOM])
                nc.vector.tensor_tensor(tf, h_b, a_b, OP.mult)
                fqg = work.tile([N, 2, G], f32, tag="fqg")
                nc.vector.tensor_reduce(fqg, tf, AX.X, OP.add)

                # f_src crosses partition->free: 8 column transposes land all
                # rows on partition 0, then a rank-1 matmul fans out to 128.
                ft_ps = ps1.tile([1, G, N], f32, tag="ft_ps")
                for g in range(G):
                    nc.tensor.transpose(ft_ps[0:1, g, :], fqg[:, 0, g:g + 1],
                                        i128_sb)
                fcat = work.tile([1, G, N], f32, tag="ftsrc")
                nc.vector.tensor_copy(fcat, ft_ps)
                e_ps = ps2.tile([N, G, N], f32, tag="gt_ps")
                for hh in range(2):
                    nc.tensor.matmul(e_ps[:, 4 * hh:4 * (hh + 1), :], ones1,
                                     fcat[:, 4 * hh:4 * (hh + 1), :],
                                     start=True, stop=True)

                # e[q, (g,p)] = f_src_g[p] + f_dst_g[q]
                e_sb = work.tile([N, G, N], f32, tag="e_sb")
                fd_b = fqg[:, 1, :].unsqueeze(2).to_broadcast([N, G, N])
                nc.vector.tensor_tensor(e_sb, e_ps, fd_b, OP.add)

                # lrelu(e) = 0.01*e + relu(0.99*e); Relu/Exp share the
                # exp_and_others ACT table set, so no per-round table loads.
                r_sb = work.tile([N, G, N], f32, tag="r_sb")
                nc.scalar.activation(out=r_sb, in_=e_sb, func=AF.Relu, scale=0.99)
                t_sb = work.tile([N, G, N], f32, tag="t_sb")
                nc.vector.tensor_scalar_mul(t_sb, e_sb, 0.01)
                nc.vector.tensor_tensor(t_sb, t_sb, r_sb, OP.add)
                p_sb = work.tile([N, G, N], f32, tag="p_sb")
                nc.scalar.activation(out=p_sb, in_=t_sb, func=AF.Exp)

                # mask multiply (split DVE / GPSIMD)
                pm = work.tile([N, G, N], f32, tag="pm")
                nc.vector.tensor_tensor(pm[:, 0:4, :], p_sb[:, 0:4, :],
                                        m01t_sb[:, 0:4, :], OP.mult)
                nc.gpsimd.tensor_tensor(pm[:, 4:G, :], p_sb[:, 4:G, :],
                                        m01t_sb[:, 4:G, :], OP.mult)

                # U = P @ [h | 1]  (last col = row-sum of P)
                u_ps = ps1.tile([N, G, ATOM + 1], f32, tag="u_ps")
                for g in range(G):
                    nc.tensor.matmul(u_ps[:, g, :], pm[:, g, :], haug[:, g, :],
                                     start=True, stop=True)

                irs = work.tile([N, G], f32, tag="irs")
                nc.vector.reciprocal(irs, u_ps[:, :, ATOM])

                # delta = U * (1/rowsum) in node-major layout
                dlt = work.tile([N, G, ATOM], f32, tag="dlt")
                i_b = irs.unsqueeze(2).to_broadcast([N, G, ATOM])
                nc.vector.tensor_tensor(dlt, u_ps[:, :, 0:ATOM], i_b, OP.mult)

                # transpose deltas per graph and accumulate into the state
                dt_ps = ps1.tile([ATOM, G, N], f32, tag="dt_ps")
                for g in range(G):
                    nc.tensor.transpose(dt_ps[:, g, :], dlt[:, g, :], i128_sb)
                nc.vector.tensor_tensor(state[0:ATOM], state[0:ATOM], dt_ps,
                                        OP.add)
                if dbg == r + 1:
                    xd = work.tile([ATOM, G, N], f32, tag="xd")
                    nc.vector.tensor_copy(xd, state[0:ATOM])
                    nc.sync.dma_start(out=d_xdbg[:], in_=xd)

            # ---- g = relu(x3 @ Wt + bt), kept transposed in two 128-halves ----
            gt_sb = []
            for hh in range(2):
                gt_ps = ps2.tile([128, G, N], f32, tag="gt_ps")
                for half in range(2):
                    sl = slice(4 * half, 4 * (half + 1))
                    nc.tensor.matmul(gt_ps[:, sl, :], wt_sb[:, hh, :],
                                     state[:, sl, :], start=True, stop=True)
                gts = singles.tile([128, G, N], f32, tag=f"gt{hh}")
                nc.scalar.activation(out=gts, in_=gt_ps, func=AF.Relu)
                gt_sb.append(gts)
            if dbg == 4:
                nc.sync.dma_start(out=d_xdbg[:], in_=gt_sb[0])

            # ---- d1 = g @ Wf (+ x0 residual after transpose; bf folded in b2n)
            d1_ps = ps1.tile([N, G, ATOM], f32, tag="h_ps")
            for g in range(G):
                nc.tensor.matmul(d1_ps[:, g, :], gt_sb[0][:, g, :], wf_sb[:, 0, :],
                                 start=True, stop=False)
                nc.tensor.matmul(d1_ps[:, g, :], gt_sb[1][:, g, :], wf_sb[:, 1, :],
                                 start=False, stop=True)
            d1n = work.tile([N, G, ATOM], f32, tag="d1n")
            nc.vector.tensor_copy(d1n, d1_ps)

            d1t_ps = ps1.tile([ATOM, G, N], f32, tag="dt_ps")
            for g in range(G):
                nc.tensor.transpose(d1t_ps[:, g, :], d1n[:, g, :], i128_sb)
            d1t_sb = work.tile([ATOM, G, N], f32, tag="d1t_sb")
            nc.vector.tensor_tensor(d1t_sb, d1t_ps, x0td, OP.add)
            if dbg == 5:
                nc.sync.dma_start(out=d_xdbg[:], in_=d1t_sb)

            # ---- d2 = d1 @ Wf2 (transposed out), then max over nodes ----
            d2_ps = ps2.tile([LATENT, G, N], f32, tag="gt_ps")
            for half in range(2):
                sl = slice(4 * half, 4 * (half + 1))
                nc.tensor.matmul(d2_ps[:, sl, :], wf2_sb, d1t_sb[:, sl, :],
                                 start=True, stop=True)
            dm = work.tile([LATENT, G], f32, tag="dm")
            nc.vector.tensor_reduce(dm, d2_ps, AX.X, OP.max)
            if dbg == 6:
                nc.sync.dma_start(out=d_xdbg[:], in_=dm)

            # ---- vec = sigmoid([dmax + b2', cell]) via exp + reciprocal ----
            v0 = work.tile([LATENT, G], f32, tag="v0")
            nc.scalar.activation(out=v0, in_=dm, func=AF.Exp, bias=b2n_sb,
                                 scale=-1.0)
            nc.vector.tensor_scalar_add(v0, v0, 1.0)
            nc.vector.reciprocal(v0, v0)
            if dbg == 7:
                nc.sync.dma_start(out=d_xdbg[:], in_=v0)

            vc_ps = ps1.tile([128, 4, G], f32, tag="u_ps")
            for c in range(4):
                nc.tensor.transpose(vc_ps[:, c, :], cl_sb[:, c * 128:(c + 1) * 128],
                                    i128_sb[0:G, 0:G])
            vc = work.tile([128, 4, G], f32, tag="vc")
            nc.scalar.activation(out=vc, in_=vc_ps, func=AF.Exp, scale=-1.0)
            nc.vector.tensor_scalar_add(vc, vc, 1.0)
            nc.vector.reciprocal(vc, vc)
            if dbg == 8:
                nc.sync.dma_start(out=d_xdbg[:], in_=vc)

            # ---- decoder MLP (graphs on the free dim) ----
            h1_ps = ps1.tile([128, G], f32, tag="ft_ps")
            nc.tensor.matmul(h1_ps, w1_sb[:, 0, :], v0, start=True, stop=False)
            for c in range(4):
                nc.tensor.matmul(h1_ps, w1_sb[:, c + 1, :], vc[:, c, :],
                                 start=False, stop=(c == 3))
            h1 = work.tile([128, G], f32, tag="h1")
            nc.vector.tensor_scalar(h1, h1_ps, b1_sb, 0.0, OP.add, OP.max)
            if dbg == 9:
                nc.sync.dma_start(out=d_xdbg[:], in_=h1)

            h2_ps = ps1.tile([128, 2, G], f32, tag="u_ps")
            for b in range(2):
                nc.tensor.matmul(h2_ps[:, b, :], w2_sb[:, b, :], h1,
                                 start=True, stop=True)
            h2 = work.tile([128, 2, G], f32, tag="h2")
            for b in range(2):
                nc.vector.tensor_scalar(h2[:, b, :], h2_ps[:, b, :],
                                        b2d_sb[:, b:b + 1], 0.0, OP.add, OP.max)

            if dbg == 10:
                nc.sync.dma_start(out=d_xdbg[:], in_=h2)
            h3_ps = ps1.tile([128, 4, G], f32, tag="ft_ps")
            for b in range(4):
                for kc in range(2):
                    nc.tensor.matmul(h3_ps[:, b, :], w3_sb[:, kc, b, :],
                                     h2[:, kc, :], start=(kc == 0),
                                     stop=(kc == 1))
            h3 = work.tile([128, 4, G], f32, tag="h3")
            for b in range(4):
                nc.vector.tensor_scalar(h3[:, b, :], h3_ps[:, b, :],
                                        b3_sb[:, b:b + 1], 0.0, OP.add, OP.max)

            if dbg == 11:
                nc.sync.dma_start(out=d_xdbg[:], in_=h3)
            s_ps = ps1.tile([1, G], f32, tag="u_ps")
            for c in range(4):
                nc.tensor.matmul(s_ps, w4_sb[:, c:c + 1], h3[:, c, :],
                                 start=(c == 0), stop=(c == 3))
            s_sb = work.tile([1, G], f32, tag="s_sb")
            if dbg == 12:
                nc.vector.tensor_copy(s_sb, s_ps)
                nc.sync.dma_start(out=d_xdbg[:], in_=s_sb)
            else:
                nc.vector.tensor_scalar_add(s_sb, s_ps, b4_sb)
                if dbg == 13:
                    nc.sync.dma_start(out=d_xdbg[:], in_=s_sb)
                nc.sync.dma_start(out=d_score.rearrange("g x -> x g"),
                                  in_=s_sb[0:1, :])

    return nc


def _fix_preamble_regs(nc):
    """Bacc defers register allocation; its alloc_regs pass skips the
    framework preamble registers (*_zero, *_bcreg*, *_tpb_base*, monotonic),
    leaving reg_id=-1 which walrus rejects. Assign collision-free ids."""
    per_engine_used = {}
    pending = []
    for alloc in nc.m.functions[0].allocations:
        eng = getattr(alloc, "engine", None)
        rid = getattr(alloc, "reg_id", None)
        if eng is None or rid is None:
            continue
        if rid >= 0:
            per_engine_used.setdefault(eng, set()).add(rid)
        else:
            pending.append(alloc)
    canonical = {"zero": 8, "monotonic_0_cnt": 9, "bcreg0_lo": 10,
                 "bcreg0_hi": 11, "bcreg1_lo": 12, "bcreg1_hi": 13,
                 "tpb_base_lo": 14, "tpb_base_hi": 15}
    for alloc in pending:
        eng = alloc.engine
        used = per_engine_used.setdefault(eng, set())
        suffix = alloc.name.split("_", 1)[1] if "_" in alloc.name else alloc.name
        rid = canonical.get(suffix, 16)
        while rid in used:
            rid += 1
        alloc.reg_id = rid
        used.add(rid)


def _stage(inputs):
    """Host-side layout staging. Returns per-core in_maps."""
    xs = np.asarray(inputs["xs"], dtype=np.float32)
    A = np.asarray(inputs["A"])
    cell = np.asarray(inputs["cell_emb"], dtype=np.float32)
    Wg = np.asarray(inputs["Wg"], dtype=np.float32)
    bg = np.asarray(inputs["bg"], dtype=np.float32)
    attn = np.asarray(inputs["attn"], dtype=np.float32)
    Wt = np.asarray(inputs["Wt"], dtype=np.float32)
    bt = np.asarray(inputs["bt"], dtype=np.float32)
    Wf = np.asarray(inputs["Wf"], dtype=np.float32)
    bf = np.asarray(inputs["bf"], dtype=np.float32)
    Wf2 = np.asarray(inputs["Wf2"], dtype=np.float32)
    bf2 = np.asarray(inputs["bf2"], dtype=np.float32)
    W1 = np.asarray(inputs["W1"], dtype=np.float32)
    b1 = np.asarray(inputs["b1"], dtype=np.float32)
    W2 = np.asarray(inputs["W2"], dtype=np.float32)
    b2 = np.asarray(inputs["b2"], dtype=np.float32)
    W3 = np.asarray(inputs["W3"], dtype=np.float32)
    b3 = np.asarray(inputs["b3"], dtype=np.float32)
    W4 = np.asarray(inputs["W4"], dtype=np.float32)
    b4 = np.asarray(inputs["b4"], dtype=np.float32)

    wg_aug = np.concatenate([Wg, bg[:, None, :]], axis=1).copy()        # [3,35,34]
    a12 = attn.reshape(3, 2, ATOM).copy()                               # [3,2,34]
    wt_aug = np.concatenate([Wt, bt[None, :]], axis=0).copy()           # [35,256]
    wf_c = Wf.reshape(2, 128, ATOM).copy()                              # [2,128,34]
    b2n = -(bf @ Wf2 + bf2).reshape(LATENT, 1).copy()                   # [128,1]
    w1_c = W1.reshape(5, 128, 128).copy()
    w2_c = np.ascontiguousarray(W2.reshape(128, 2, 128).transpose(1, 0, 2))
    b2d_c = b2.reshape(2, 128, 1).copy()
    w3_c = np.ascontiguousarray(
        W3.reshape(2, 128, 4, 128).transpose(0, 2, 1, 3))               # [kc,b,128,128]
    b3_c = b3.reshape(4, 128, 1).copy()
    w4_c = W4.reshape(4, 128, 1).copy()
    b4_c = b4.reshape(1, 1).copy()
    i128 = np.eye(128, dtype=np.float32)

    shared = dict(wg=wg_aug, a12=a12, wt=wt_aug, wf=wf_c, wf2=Wf2.copy(),
                  b2n=b2n, w1=w1_c, b1=b1.reshape(128, 1).copy(), w2=w2_c,
                  b2d=b2d_c, w3=w3_c, b3=b3_c, w4=w4_c, b4=b4_c, i128=i128)

    in_maps = []
    for core in range(NCORES):
        sl = slice(core * G, (core + 1) * G)
        x0t = np.concatenate(
            [xs[sl].transpose(0, 2, 1),
             np.ones((G, 1, N), np.float32)], axis=1).copy()     # [8,35,128]
        m01t = np.ascontiguousarray(
            (A[sl] > 0).transpose(0, 2, 1).astype(np.float32))   # [8,128,128]
        m = dict(shared)
        m.update(x0t=x0t, m01t=m01t, cell=np.ascontiguousarray(cell[sl]))
        in_maps.append(m)
    return in_maps


def get_nc(dbg=0):
    key = f"nc{dbg}"
    if key not in _CACHE:
        nc = _build_bass(dbg)
        nc.finalize()
        _fix_preamble_regs(nc)
        _CACHE[key] = nc
    return _CACHE[key]


def kernel(**inputs) -> np.ndarray:
    from concourse.bass_utils import run_bass_kernel_spmd

    nc = get_nc()
    in_maps = _stage(inputs)
    res = run_bass_kernel_spmd(nc, in_maps, core_ids=list(range(NCORES)))
    out = np.concatenate([res.results[i]["score"] for i in range(NCORES)], axis=0)
    return out.astype(np.float32)

